# revision 68
# baseline (speedup 1.0000x reference)
"""Causal multi-head attention (B=4, T=2048, C=384, 6 heads of 64) on 8 trn2 cores.

Sharding: 24 (batch, head) pairs -> 8 cores; core c handles batch c//2 and
heads [3*(c%2), 3*(c%2)+3). Each core computes q/k/v projections for its 3
heads, causal softmax(q k^T / 8) v, and a PARTIAL output projection
ctx_heads @ Wo_heads. Host sums the two partials per batch and adds the
exactly-folded bias terms (bv @ Wo + bo; softmax weights sum to 1 so a v-bias
contributes bv @ Wo to every row).

v2: streaming schedule. Projections are chunked per 512-wide t-chunk and
interleaved into the attention loop (attention for t-chunk tci only needs
projections of chunks <= tci). The sb loop is software-pipelined so the
scalar engine (exp, the bottleneck) never starves. Diagonal blocks use
dedicated e-tiles with memset-zero prefixes so exp/masks only cover the
valid triangle. Softmax sums stay on-chip (no DRAM round trip) and use
reciprocal_approx_fast.

Requires bq == bk == 0 (true for this problem: spec fill=zeros).
"""

import math
import os
from contextlib import ExitStack

import ml_dtypes
import numpy as np

BF16NP = ml_dtypes.bfloat16

B, T, C = 4, 2048, 384
NH, D = 6, 64          # total heads, head dim
HPC = 3                # heads per core
NCORES = 8
NKC = C // 128         # 3 contraction chunks for the projections
NTB = T // 128         # 16 row blocks
TCW = 512              # t-chunk width for the attention loop
NTC = T // TCW         # 4 t-chunks

_CACHED_NC = None

# names for the packed q/k projections; chunked per 512-wide t-range.
# wqk columns: mt0=[q0|q1] mt1=[k0|k1] mt2=[q2|q2] mt3=[k2|k2]
# (q columns pre-scaled by 1/sqrt(D) on host)
QK_NAMES = ["qT01", "kT01", "qT22", "kT22"]


def build_nc():
    import concourse.bass as bass
    import concourse.mybir as mybir
    import concourse.tile as tile
    from concourse import bacc

    F32 = mybir.dt.float32
    BF16 = mybir.dt.bfloat16
    EXPF = mybir.ActivationFunctionType.Exp

    nc = bacc.Bacc("TRN2", target_bir_lowering=False, debug=False)

    xt = nc.dram_tensor("xt", [C, T], BF16, kind="ExternalInput")
    wqk = nc.dram_tensor("wqk", [C, 512], BF16, kind="ExternalInput")
    wv = nc.dram_tensor("wv", [C, 192], BF16, kind="ExternalInput")
    wo = nc.dram_tensor("wo", [HPC * D, 384], BF16, kind="ExternalInput")
    zt = nc.dram_tensor("zt", [128, 512], BF16, kind="ExternalInput")
    tri2 = nc.dram_tensor("tri2", [128, 256], BF16, kind="ExternalInput")
    out = nc.dram_tensor("out", [T, C], F32, kind="ExternalOutput")
    srow = nc.dram_tensor("srow", [NTC * HPC, TCW], F32)  # sums rows scratch

    with ExitStack() as ctx:
        tc = ctx.enter_context(tile.TileContext(nc))
        const = ctx.enter_context(tc.tile_pool(name="const", bufs=1))
        xpool = ctx.enter_context(tc.tile_pool(name="xp", bufs=1))
        qkpool = ctx.enter_context(tc.tile_pool(name="qkp", bufs=1))
        vpool = ctx.enter_context(tc.tile_pool(name="vp", bufs=1))
        expp = ctx.enter_context(tc.tile_pool(name="expp", bufs=6))
        diagp = ctx.enter_context(tc.tile_pool(name="diagp", bufs=1))
        cxp = ctx.enter_context(tc.tile_pool(name="cxp", bufs=1))
        rpool = ctx.enter_context(tc.tile_pool(name="rp", bufs=2))
        opool = ctx.enter_context(tc.tile_pool(name="op", bufs=2))
        ps_s = ctx.enter_context(tc.tile_pool(name="ps_s", bufs=2, space="PSUM"))
        ps_c = ctx.enter_context(tc.tile_pool(name="ps_c", bufs=1, space="PSUM"))
        ps_o = ctx.enter_context(tc.tile_pool(name="ps_o", bufs=1, space="PSUM"))



        # ---- DMA queue layout: one kc-row of x + its weights per DMA
        # queue (sync/scalar HW-DGE, gpsimd SW-DGE), ordered by first use:
        # wqk -> x chunk 0 -> wv -> later x chunks. x is 12 [128,512]
        # tiles so the first projections only wait on 128KB each. ----
        xts = [[None] * 4 for _ in range(NKC)]
        for kc in range(NKC):
            for nch in range(4):
                xts[kc][nch] = xpool.tile(
                    [128, 512], BF16, tag=f"xt{kc}_{nch}", name=f"x{kc}_{nch}"
                )
        wqk_sb = [
            const.tile([128, 512], BF16, tag=f"wqk{kc}", name=f"wqk{kc}")
            for kc in range(NKC)
        ]
        tri2_sb = const.tile([128, 2, 128], BF16, tag="tri2")
        zt_sb = const.tile([128, 512], BF16, tag="zt")
        wv_sb = [
            const.tile([128, 192], BF16, tag=f"wv{kc}", name=f"wv{kc}")
            for kc in range(NKC)
        ]
        wo_sb = [
            const.tile([64, 384], BF16, tag=f"wo{h}", name=f"wo{h}")
            for h in range(HPC)
        ]

        def ldx(eng, kc, nch):
            eng.dma_start(
                out=xts[kc][nch],
                in_=xt[kc * 128:(kc + 1) * 128, nch * 512:(nch + 1) * 512],
            )

        def ldw(eng, dst, src):
            eng.dma_start(out=dst, in_=src)

        # Per-queue DMA is ~85GB/s with ~2.5us spin-up, so the startup-
        # critical set (wqk + x chunk 0 + wv) spreads across ALL three
        # queues. Scalar-queue issues cost the ACT engine ~800ns each, but
        # ACT is idle at startup -- only these three ride there.
        ldw(nc.sync, wqk_sb[0], wqk[0:128, :])
        ldx(nc.sync, 0, 0)
        ldw(nc.scalar, wqk_sb[1], wqk[128:256, :])
        ldw(nc.scalar, tri2_sb, tri2.rearrange("p (two w) -> p two w", two=2))
        ldx(nc.scalar, 1, 0)
        ldw(nc.gpsimd, wqk_sb[2], wqk[256:384, :])
        ldx(nc.gpsimd, 2, 0)
        ldw(nc.sync, wv_sb[0], wv[0:128, :])
        ldw(nc.gpsimd, wv_sb[1], wv[128:256, :])
        ldw(nc.gpsimd, wv_sb[2], wv[256:384, :])
        ldw(nc.gpsimd, zt_sb, zt[:, :])
        for nch in range(1, 4):
            ldx(nc.sync, 0, nch)
            ldx(nc.sync, 1, nch)
            ldx(nc.gpsimd, 2, nch)
        for h in range(HPC):
            ldw(nc.gpsimd, wo_sb[h], wo[h * 64:(h + 1) * 64, :])

        def xv(kc, nch):
            return xts[kc][nch][:, :]

        # ones row at partition 64 (matches the sums row of the cues) for
        # the tail's K=1 broadcast matmuls
        ones_row = const.tile([65, 64], F32, tag="ones_row")
        nc.vector.memset(ones_row[64:65, :], 1.0)

        # ---- ACT exp-table preload AFTER the scalar-queue DMA issues (the
        # ~2.7us table load must not delay them); still well before the
        # first real exp. ----
        warm = const.tile([1, 8], F32, tag="warm")
        nc.vector.memset(warm[:, :], 0.0)
        nc.scalar.activation(warm[:, :], warm[:, :], EXPF)


        # ---- dedicated diagonal e-tiles with persistent zero prefixes ----
        # diag01[j]: e01 tile for the j-th diagonal s-block of any t-chunk
        # (mask prefix = 128*j cols in each 512-half; zeroed once, exp only
        # ever writes the valid suffix).
        diag01 = []
        for j in range(4):
            dt_ = diagp.tile([128, 2, 512], BF16, tag=f"d01_{j}")
            if j > 0:
                nc.vector.memset(dt_[:, :, 0:128 * j], 0.0)
            diag01.append(dt_)

        # ---- per-chunk projection tiles ----
        qkT = {name: [None] * 4 for name in QK_NAMES}
        for nch in range(4):
            for name in QK_NAMES:
                qkT[name][nch] = qkpool.tile(
                    [128, 512], BF16, tag=f"{name}_{nch}", name=f"{name}_{nch}"
                )
        # v chunks: [s(128), tb within chunk, head, 66] with ones cols 64:66
        v4 = []
        for nch in range(4):
            vt = vpool.tile([128, 4, HPC, 66], BF16, tag=f"v{nch}")
            nc.vector.memset(vt[:, :, :, 64:66], 1.0)
            v4.append(vt)

        # per-head per-chunk normalized ctx^T [64, 512]
        ctxT = [
            [
                cxp.tile([64, TCW], BF16, tag=f"cT{h}_{tci}", name=f"cT{h}_{tci}")
                for tci in range(NTC)
            ]
            for h in range(HPC)
        ]
        # ctx psums: one stable bank per head
        cps = [
            ps_c.tile([128, TCW], F32, tag=f"cps{h}", name=f"cps{h}")
            for h in range(HPC)
        ]

        # ---------- emission pieces ----------
        def qk_piece(nch, mt):
            ps = ps_s.tile([128, 2, 512], F32, tag="S")
            for kc in range(NKC):
                nc.tensor.matmul(
                    ps[:, 0, :],
                    lhsT=wqk_sb[kc][:, mt * 128:(mt + 1) * 128],
                    rhs=xv(kc, nch),
                    start=(kc == 0),
                    stop=(kc == NKC - 1),
                )
            nc.vector.tensor_copy(out=qkT[QK_NAMES[mt]][nch][:, :], in_=ps[:, 0, :])

        def v_piece(nch, i):
            tb = 4 * nch + i
            ps = ps_s.tile([128, 2, 512], F32, tag="S")
            for kc in range(NKC):
                nc.tensor.matmul(
                    ps[:, 0, 0:192],
                    lhsT=xv(kc, nch)[:, i * 128:(i + 1) * 128],
                    rhs=wv_sb[kc][:, :],
                    start=(kc == 0),
                    stop=(kc == NKC - 1),
                )
            dst = v4[nch][:, i, :, 0:64]
            src = ps[:, 0, 0:192].rearrange("p (h e) -> p h e", e=64)
            nc.vector.tensor_copy(out=dst, in_=src)

        def proj_pieces(nch, order=(0, 2, 1, 3)):
            # for nch>=1 (fed as extras): mt0 (qT01) and mt2 (qT22) gate the
            # NEXT chunk's first pair; mt1/mt3 (kT chunks) and v only gate
            # its last two pairs. For nch=0 natural order (scores01 pair0
            # needs mt0+mt1 first).
            ps_list = [lambda mt=mt: qk_piece(nch, mt) for mt in order]
            ps_list += [lambda i=i: v_piece(nch, i) for i in range(4)]
            return ps_list

        def scores01(tci, sb):
            """Packed h0/h1 scores for s-block sb -> psum [128, 2, 512]."""
            off = sb * 128 - tci * TCW  # >=0 iff diagonal block
            lo = max(off, 0)
            ps = ps_s.tile([128, 2, 512], F32, tag="S")
            kch, kcol = sb // 4, (sb % 4) * 128
            for hh in range(2):
                psl = slice(hh * 64, (hh + 1) * 64)
                nc.tensor.matmul(
                    ps[:, hh, lo:512],
                    lhsT=qkT["kT01"][kch][psl, kcol:kcol + 128],
                    rhs=qkT["qT01"][tci][psl, lo:512],
                    start=True,
                    stop=True,
                )
            return ps, lo

        def scores2(tci, sb):
            """Packed h2 scores for s-blocks (sb, sb+1) -> psum [128,2,512]."""
            ps = ps_s.tile([128, 2, 512], F32, tag="S")
            los = []
            for j in range(2):
                sbj = sb + j
                off = sbj * 128 - tci * TCW
                lo = max(off, 0)
                los.append(lo)
                kch, kcol = sbj // 4, (sbj % 4) * 128
                psl = slice(j * 64, (j + 1) * 64)
                # full width: the exp reads the whole tile, so every column
                # must be written this generation (masked-out cols get zeroed
                # by the wide zt mask after exp)
                nc.tensor.matmul(
                    ps[:, j, :],
                    lhsT=qkT["kT22"][kch][psl, kcol:kcol + 128],
                    rhs=qkT["qT22"][tci][psl, :],
                    start=True,
                    stop=True,
                )
            return ps, los

        def ctx01(tci, sb, e_t, nsb, lo=0):
            # lo>0 on diagonal blocks: cols [0:lo) of e are memset zeros --
            # skip them (saves tensor streaming cycles; psum cols [0:lo)
            # keep their accumulated value)
            for hh in range(2):
                nc.tensor.matmul(
                    cps[hh][0:66, lo:512],
                    lhsT=v4[sb // 4][:, sb % 4, hh, :],
                    rhs=e_t[:, hh, lo:512],
                    start=(sb == 0),
                    stop=(sb == nsb - 1),
                )

        def ctx2(tci, sb, e_t, nsb, los=(0, 0)):
            for j in range(2):
                sbj = sb + j
                lo = los[j]
                nc.tensor.matmul(
                    cps[2][0:66, lo:512],
                    lhsT=v4[sbj // 4][:, sbj % 4, 2, :],
                    rhs=e_t[:, j, lo:512],
                    start=(sbj == 0),
                    stop=(sbj == nsb - 1),
                )

        cue_t = {}

        def epi_cue(tci, h):
            """Evac ctx+sums for head h (frees the cps bank for the next
            chunk). Emitted in-chunk right after the head's final ctx. At
            the tail only h2's cue follows the last exp, so it rides the
            (now idle) scalar engine; h0/h1 overlap exp2ab on vector."""
            cue = rpool.tile([66, TCW], F32, tag=f"cue{h}", name=f"cue{tci}_{h}")
            if tci == 3 and h == 2:
                nc.scalar.copy(out=cue[:, :], in_=cps[h][0:66, :])
            else:
                nc.vector.tensor_copy(out=cue[:, :], in_=cps[h][0:66, :])
            cue_t[(tci, h)] = cue

        def epi_fin(tci, h):
            """Normalize into ctxT. Mid-kernel: broadcast sums via a DRAM
            round-trip DMA (zero tensor-engine cost, latency hides under
            attention). Tail (tci=3): the latency is exposed, so use a K=1
            broadcast matmul into the freed cps bank instead (TE is idle)."""
            cue = cue_t.pop((tci, h))
            if tci == 3:
                psB = ps_c.tile([128, TCW], F32, tag=f"cps{h}", name=f"bc{tci}_{h}")
                nc.tensor.matmul(
                    psB[0:64, :],
                    lhsT=ones_row[64:65, :],
                    rhs=cue[64:65, :],
                    start=True,
                    stop=True,
                )
                sb_src = psB[0:64, :]
            else:
                idx = tci * HPC + h
                nc.sync.dma_start(out=srow[idx:idx + 1, :], in_=cue[64:65, :])
                sb_b = rpool.tile([64, TCW], F32, tag=f"sb{h}", name=f"sb{tci}_{h}")
                nc.sync.dma_start(
                    out=sb_b[:, :],
                    in_=srow[idx:idx + 1, :].to_broadcast([64, TCW]),
                )
                sb_src = sb_b[:, :]
            rec = rpool.tile([64, TCW], F32, tag=f"rec{h}", name=f"rec{tci}_{h}")
            nc.vector.reciprocal_approx_fast(out=rec[:, :], in_=sb_src)
            nc.vector.tensor_mul(ctxT[h][tci][:, :], cue[0:64, :], rec[:, :])

        def po_piece(tci, i, tag="O"):
            """Output projection for row block tb = 4*tci + i."""
            tb = 4 * tci + i
            if tag == "O":
                po = ps_o.tile([128, 512], F32, tag="O", name=f"po{tb}")
            else:
                po = ps_c.tile([128, TCW], F32, tag=tag, name=f"po{tb}")
            for h in range(HPC):
                nc.tensor.matmul(
                    po[:, 0:384],
                    lhsT=ctxT[h][tci][:, i * 128:(i + 1) * 128],
                    rhs=wo_sb[h][:, :],
                    start=(h == 0),
                    stop=(h == HPC - 1),
                )
            osb = opool.tile([128, 384], F32, tag="osb", name=f"osb{tb}")
            # tail (tci=3): scalar engine is idle (exps done) and the sync
            # queue is free -- route the evac + store off the busy engines
            if tci == 3:
                # alternate the evac between the two psum-capable engines
                if i % 2 == 0:
                    nc.scalar.copy(out=osb[:, :], in_=po[:, 0:384])
                else:
                    nc.vector.tensor_copy(out=osb[:, :], in_=po[:, 0:384])
                nc.sync.dma_start(out=out[tb * 128:(tb + 1) * 128, :], in_=osb[:, :])
            else:
                nc.vector.tensor_copy(out=osb[:, :], in_=po[:, 0:384])
                nc.gpsimd.dma_start(out=out[tb * 128:(tb + 1) * 128, :], in_=osb[:, :])

        def attn(tci, extras, s_carry):
            """Attention for t-chunk tci; pops `extras` (closures) into the
            emission stream to fill tensor-engine slack. scores01 for each
            pair is hoisted one pair ahead (including across the chunk
            boundary via s_carry) so the scalar engine never waits on TE
            queue position. Returns the next chunk's hoisted first scores."""
            nsb = 4 * tci + 4
            npair = nsb // 2
            ex_i = 0
            slots_left = 3 * npair

            def pop_extras():
                nonlocal ex_i, slots_left
                n_left = len(extras) - ex_i
                k = -(-n_left // max(slots_left, 1))
                slots_left -= 1
                for _ in range(k):
                    if ex_i < len(extras):
                        extras[ex_i]()
                        ex_i += 1

            for p in range(npair):
                sb, sbb = 2 * p, 2 * p + 1
                diag_a = sb >= 4 * tci
                diag_b = sbb >= 4 * tci
                last = p == npair - 1

                # scores h0/h1 for sb (hoisted into the previous pair)
                if s_carry is not None:
                    s_a, lo_a = s_carry
                else:
                    s_a, lo_a = scores01(tci, sb)
                # exp(sb) -- into dedicated diag tile or rotating tile
                if diag_a:
                    e_a = diag01[sb - 4 * tci]
                    nc.scalar.activation(
                        e_a[:, :, lo_a:512], s_a[:, :, lo_a:512], EXPF
                    )
                else:
                    e_a = expp.tile([128, 2, 512], BF16, tag="E")
                    nc.scalar.activation(e_a[:, :, :], s_a[:, :, :], EXPF)
                # scores h0/h1 for sbb (runs on TE while exp(sb) runs on ACT)
                s_b, lo_b = scores01(tci, sbb)
                pop_extras()
                if diag_a:
                    nc.vector.tensor_mul(
                        e_a[:, :, lo_a:lo_a + 128],
                        e_a[:, :, lo_a:lo_a + 128],
                        tri2_sb[:, :, :],
                    )
                ctx01(tci, sb, e_a, nsb, lo=lo_a)
                # exp(sbb)
                if diag_b:
                    e_b = diag01[sbb - 4 * tci]
                    nc.scalar.activation(
                        e_b[:, :, lo_b:512], s_b[:, :, lo_b:512], EXPF
                    )
                else:
                    e_b = expp.tile([128, 2, 512], BF16, tag="E")
                    nc.scalar.activation(e_b[:, :, :], s_b[:, :, :], EXPF)
                # h2 scores for the pair (TE during exp(sbb))
                s_2, los2 = scores2(tci, sb)
                # hoist the NEXT pair's (or next chunk's) h0/h1 scores
                if not last:
                    s_carry = scores01(tci, sb + 2)
                elif tci + 1 < NTC:
                    s_carry = scores01(tci + 1, 0)
                else:
                    s_carry = None
                pop_extras()
                if diag_b:
                    nc.vector.tensor_mul(
                        e_b[:, :, lo_b:lo_b + 128],
                        e_b[:, :, lo_b:lo_b + 128],
                        tri2_sb[:, :, :],
                    )
                ctx01(tci, sbb, e_b, nsb, lo=lo_b)
                if last:
                    epi_cue(tci, 0)
                    epi_cue(tci, 1)
                    if tci == 3:
                        # normalize h0 while the final h2 exp runs
                        epi_fin(tci, 0)
                # exp h2 (full width; stale prefix cols get masked below)
                e_2 = expp.tile([128, 2, 512], BF16, tag="E")
                nc.scalar.activation(e_2[:, :, :], s_2[:, :, :], EXPF)
                for j in range(2):
                    if sb + j >= 4 * tci:  # diagonal: wide zt mask
                        w = los2[j] + 128
                        nc.vector.tensor_mul(
                            e_2[:, j, 0:w], e_2[:, j, 0:w], zt_sb[:, 512 - w:512]
                        )
                if last and tci == 3:
                    epi_fin(tci, 1)
                ctx2(tci, sb, e_2, nsb, los=[
                    los2[j] if sb + j >= 4 * tci else 0 for j in range(2)
                ])
                if last:
                    epi_cue(tci, 2)
                pop_extras()
            while ex_i < len(extras):  # any stragglers
                extras[ex_i]()
                ex_i += 1
            return s_carry

        # ---------- top-level schedule ----------
        for piece in proj_pieces(0, order=(0, 1, 2, 3)):
            piece()

        def fin_pieces(tci):
            return [lambda h=h: epi_fin(tci, h) for h in range(HPC)]

        def po_pieces(tci):
            return [lambda i=i: po_piece(tci, i) for i in range(4)]

        sc = attn(0, proj_pieces(1), None)
        sc = attn(1, fin_pieces(0) + po_pieces(0) + proj_pieces(2), sc)
        sc = attn(2, fin_pieces(1) + po_pieces(1) + proj_pieces(3), sc)
        sc = attn(3, fin_pieces(2) + po_pieces(2), sc)
        # tail: finish chunk 3's h2 normalize (h0/h1 ran inside the last
        # pair), then the last 4 output projections across the freed ctx
        # psum banks + the O bank -- in two phases so the h0/h1 matmuls
        # (whose ctxT is ready ~3us before h2's) run during the h2
        # normalize chain, and only the four h2 finishers wait on mul2.
        epi_fin(3, 2)
        po_tail = []
        for i, tg in enumerate(["O", "cps0", "cps1", "cps2"]):
            if tg == "O":
                po = ps_o.tile([128, 512], F32, tag="O", name=f"pot{i}")
            else:
                po = ps_c.tile([128, TCW], F32, tag=tg, name=f"pot{i}")
            po_tail.append(po)
            for h in range(2):
                nc.tensor.matmul(
                    po[:, 0:384],
                    lhsT=ctxT[h][3][:, i * 128:(i + 1) * 128],
                    rhs=wo_sb[h][:, :],
                    start=(h == 0),
                    stop=False,
                )
        for i in range(4):
            tb = 12 + i
            nc.tensor.matmul(
                po_tail[i][:, 0:384],
                lhsT=ctxT[2][3][:, i * 128:(i + 1) * 128],
                rhs=wo_sb[2][:, :],
                start=False,
                stop=True,
            )
            osb = opool.tile([128, 384], F32, tag="osb", name=f"osbt{tb}")
            if i % 2 == 0:
                nc.scalar.copy(out=osb[:, :], in_=po_tail[i][:, 0:384])
            else:
                nc.vector.tensor_copy(out=osb[:, :], in_=po_tail[i][:, 0:384])
            nc.sync.dma_start(out=out[tb * 128:(tb + 1) * 128, :], in_=osb[:, :])

    return nc


def get_nc():
    global _CACHED_NC
    if _CACHED_NC is None:
        nc = build_nc()
        nc.finalize()
        _CACHED_NC = nc
    return _CACHED_NC


def make_core_inputs(x, Wq, bq, Wk, bk, Wv, bv, Wo, bo):
    """Host-side shard prep. Returns (in_maps, host_add) where host_add[384]
    is added to every output row (exact fold of bv/bo)."""
    scale = 1.0 / math.sqrt(D)
    assert np.all(bq == 0.0) and np.all(bk == 0.0), "kernel assumes bq=bk=0"
    host_add = (bv.astype(np.float64) @ Wo.astype(np.float64) + bo).astype(np.float32)

    si = np.arange(128)[:, None]
    tj = np.arange(128)[None, :]
    tri = (si <= tj).astype(np.float32)
    zt = np.zeros((128, 512), dtype=np.float32)
    zt[:, 384:512] = tri
    tri2 = np.concatenate([tri, tri], axis=1)

    in_maps = []
    for core in range(NCORES):
        b = core // 2
        h0 = HPC * (core % 2)  # first head (0 or 3)
        cs = slice(h0 * D, (h0 + HPC) * D)
        wq_s = (Wq[:, cs] * scale).astype(np.float32)
        wk_s = Wk[:, cs].astype(np.float32)
        wqk = np.concatenate(
            [
                wq_s[:, 0:128],
                wk_s[:, 0:128],
                np.tile(wq_s[:, 128:192], (1, 2)),
                np.tile(wk_s[:, 128:192], (1, 2)),
            ],
            axis=1,
        )
        in_maps.append(
            {
                "xt": np.ascontiguousarray(x[b].T).astype(BF16NP),
                "wqk": np.ascontiguousarray(wqk).astype(BF16NP),
                "wv": np.ascontiguousarray(Wv[:, cs]).astype(BF16NP),
                "wo": np.ascontiguousarray(Wo[cs, :]).astype(BF16NP),
                "zt": zt.astype(BF16NP),
                "tri2": tri2.astype(BF16NP),
            }
        )
    return in_maps, host_add


def kernel(x, Wq, bq, Wk, bk, Wv, bv, Wo, bo, _trace=False):
    x = np.asarray(x, dtype=np.float32)
    Wq, bq = np.asarray(Wq, np.float32), np.asarray(bq, np.float32)
    Wk, bk = np.asarray(Wk, np.float32), np.asarray(bk, np.float32)
    Wv, bv = np.asarray(Wv, np.float32), np.asarray(bv, np.float32)
    Wo, bo = np.asarray(Wo, np.float32), np.asarray(bo, np.float32)

    from concourse.bass_utils import run_bass_kernel_spmd

    nc = get_nc()
    in_maps, host_add = make_core_inputs(x, Wq, bq, Wk, bk, Wv, bv, Wo, bo)
    res = run_bass_kernel_spmd(
        nc, in_maps, core_ids=list(range(NCORES)), trace=_trace
    )
    out = np.empty((B, T, C), dtype=np.float32)
    for b in range(B):
        out[b] = res.results[2 * b]["out"] + res.results[2 * b + 1]["out"] + host_add
    if _trace:
        return out, res
    return out


# revision 70
# speedup vs baseline: 1.0393x; 1.0393x over previous
"""Causal multi-head attention (B=4, T=2048, C=384, 6 heads of 64) on 8 trn2 cores.

Sharding: 24 (batch, head) pairs -> 8 cores; core c handles batch c//2 and
heads [3*(c%2), 3*(c%2)+3). Each core computes q/k/v projections for its 3
heads, causal softmax(q k^T / 8) v, and a PARTIAL output projection
ctx_heads @ Wo_heads. Host sums the two partials per batch and adds the
exactly-folded bias terms (bv @ Wo + bo; softmax weights sum to 1 so a v-bias
contributes bv @ Wo to every row).

v2: streaming schedule. Projections are chunked per 512-wide t-chunk and
interleaved into the attention loop (attention for t-chunk tci only needs
projections of chunks <= tci). The sb loop is software-pipelined so the
scalar engine (exp, the bottleneck) never starves. Diagonal blocks use
dedicated e-tiles with memset-zero prefixes so exp/masks only cover the
valid triangle. Softmax sums stay on-chip (no DRAM round trip) and use
reciprocal_approx_fast.

Requires bq == bk == 0 (true for this problem: spec fill=zeros).
"""

import math
import os
from contextlib import ExitStack

import ml_dtypes
import numpy as np

BF16NP = ml_dtypes.bfloat16

B, T, C = 4, 2048, 384
NH, D = 6, 64          # total heads, head dim
HPC = 3                # heads per core
NCORES = 8
NKC = C // 128         # 3 contraction chunks for the projections
NTB = T // 128         # 16 row blocks
TCW = 512              # t-chunk width for the attention loop
NTC = T // TCW         # 4 t-chunks

_CACHED_NC = None

# names for the packed q/k projections; chunked per 512-wide t-range.
# wqk columns: mt0=[q0|q1] mt1=[k0|k1] mt2=[q2|q2] mt3=[k2|k2]
# (q columns pre-scaled by 1/sqrt(D) on host)
QK_NAMES = ["qT01", "kT01", "qT22", "kT22"]


def build_nc():
    import concourse.bass as bass
    import concourse.mybir as mybir
    import concourse.tile as tile
    from concourse import bacc

    F32 = mybir.dt.float32
    BF16 = mybir.dt.bfloat16
    EXPF = mybir.ActivationFunctionType.Exp

    nc = bacc.Bacc("TRN2", target_bir_lowering=False, debug=False)

    xt = nc.dram_tensor("xt", [C, T], BF16, kind="ExternalInput")
    wqk = nc.dram_tensor("wqk", [C, 512], BF16, kind="ExternalInput")
    wv = nc.dram_tensor("wv", [C, 192], BF16, kind="ExternalInput")
    wo = nc.dram_tensor("wo", [HPC * D, 384], BF16, kind="ExternalInput")
    zt = nc.dram_tensor("zt", [128, 512], BF16, kind="ExternalInput")
    tri2 = nc.dram_tensor("tri2", [128, 256], BF16, kind="ExternalInput")
    out = nc.dram_tensor("out", [T, C], F32, kind="ExternalOutput")
    srow = nc.dram_tensor("srow", [NTC * HPC, TCW], F32)  # sums rows scratch

    with ExitStack() as ctx:
        tc = ctx.enter_context(tile.TileContext(nc))
        const = ctx.enter_context(tc.tile_pool(name="const", bufs=1))
        xpool = ctx.enter_context(tc.tile_pool(name="xp", bufs=1))
        qkpool = ctx.enter_context(tc.tile_pool(name="qkp", bufs=1))
        vpool = ctx.enter_context(tc.tile_pool(name="vp", bufs=1))
        expp = ctx.enter_context(tc.tile_pool(name="expp", bufs=6))
        diagp = ctx.enter_context(tc.tile_pool(name="diagp", bufs=1))
        cxp = ctx.enter_context(tc.tile_pool(name="cxp", bufs=1))
        rpool = ctx.enter_context(tc.tile_pool(name="rp", bufs=2))
        opool = ctx.enter_context(tc.tile_pool(name="op", bufs=2))
        ps_s = ctx.enter_context(tc.tile_pool(name="ps_s", bufs=2, space="PSUM"))
        ps_c = ctx.enter_context(tc.tile_pool(name="ps_c", bufs=1, space="PSUM"))
        ps_o = ctx.enter_context(tc.tile_pool(name="ps_o", bufs=1, space="PSUM"))



        # ---- DMA queue layout: one kc-row of x + its weights per DMA
        # queue (sync/scalar HW-DGE, gpsimd SW-DGE), ordered by first use:
        # wqk -> x chunk 0 -> wv -> later x chunks. x is 12 [128,512]
        # tiles so the first projections only wait on 128KB each. ----
        xts = [[None] * 4 for _ in range(NKC)]
        for kc in range(NKC):
            for nch in range(4):
                xts[kc][nch] = xpool.tile(
                    [128, 512], BF16, tag=f"xt{kc}_{nch}", name=f"x{kc}_{nch}"
                )
        wqk_sb = [
            const.tile([128, 512], BF16, tag=f"wqk{kc}", name=f"wqk{kc}")
            for kc in range(NKC)
        ]
        tri2_sb = const.tile([128, 2, 128], BF16, tag="tri2")
        zt_sb = const.tile([128, 512], BF16, tag="zt")
        wv_sb = [
            const.tile([128, 192], BF16, tag=f"wv{kc}", name=f"wv{kc}")
            for kc in range(NKC)
        ]
        wo_sb = [
            const.tile([64, 384], BF16, tag=f"wo{h}", name=f"wo{h}")
            for h in range(HPC)
        ]

        def ldx(eng, kc, nch):
            eng.dma_start(
                out=xts[kc][nch],
                in_=xt[kc * 128:(kc + 1) * 128, nch * 512:(nch + 1) * 512],
            )

        def ldw(eng, dst, src):
            eng.dma_start(out=dst, in_=src)

        # Per-queue DMA is ~85GB/s with ~2.5us spin-up, so the startup-
        # critical set (wqk + x chunk 0 + wv) spreads across ALL three
        # queues. Scalar-queue issues cost the ACT engine ~800ns each, but
        # ACT is idle at startup -- only these three ride there.
        ldw(nc.sync, wqk_sb[0], wqk[0:128, :])
        ldx(nc.sync, 0, 0)
        ldw(nc.scalar, wqk_sb[1], wqk[128:256, :])
        ldw(nc.scalar, tri2_sb, tri2.rearrange("p (two w) -> p two w", two=2))
        ldx(nc.scalar, 1, 0)
        ldw(nc.gpsimd, wqk_sb[2], wqk[256:384, :])
        ldx(nc.gpsimd, 2, 0)
        ldw(nc.sync, wv_sb[0], wv[0:128, :])
        ldw(nc.gpsimd, wv_sb[1], wv[128:256, :])
        ldw(nc.gpsimd, wv_sb[2], wv[256:384, :])
        ldw(nc.gpsimd, zt_sb, zt[:, :])
        for nch in range(1, 4):
            ldx(nc.sync, 0, nch)
            ldx(nc.sync, 1, nch)
            ldx(nc.gpsimd, 2, nch)
        for h in range(HPC):
            ldw(nc.gpsimd, wo_sb[h], wo[h * 64:(h + 1) * 64, :])

        def xv(kc, nch):
            return xts[kc][nch][:, :]

        # ones row at partition 64 (matches the sums row of the cues) for
        # the tail's K=1 broadcast matmuls
        ones_row = const.tile([65, 64], F32, tag="ones_row")
        nc.vector.memset(ones_row[64:65, :], 1.0)

        # ---- ACT exp-table preload AFTER the scalar-queue DMA issues (the
        # ~2.7us table load must not delay them); still well before the
        # first real exp. ----
        warm = const.tile([1, 8], F32, tag="warm")
        nc.vector.memset(warm[:, :], 0.0)
        nc.scalar.activation(warm[:, :], warm[:, :], EXPF)


        # ---- dedicated diagonal e-tiles with persistent zero prefixes ----
        # diag01[j]: e01 tile for the j-th diagonal s-block of any t-chunk
        # (mask prefix = 128*j cols in each 512-half; zeroed once, exp only
        # ever writes the valid suffix).
        diag01 = []
        for j in range(4):
            dt_ = diagp.tile([128, 2, 512], BF16, tag=f"d01_{j}")
            if j > 0:
                nc.vector.memset(dt_[:, :, 0:128 * j], 0.0)
            diag01.append(dt_)

        # ---- per-chunk projection tiles ----
        qkT = {name: [None] * 4 for name in QK_NAMES}
        for nch in range(4):
            for name in QK_NAMES:
                qkT[name][nch] = qkpool.tile(
                    [128, 512], BF16, tag=f"{name}_{nch}", name=f"{name}_{nch}"
                )
        # v chunks: [s(128), tb within chunk, head, 66] with ones cols 64:66
        v4 = []
        for nch in range(4):
            vt = vpool.tile([128, 4, HPC, 66], BF16, tag=f"v{nch}")
            nc.vector.memset(vt[:, :, :, 64:66], 1.0)
            v4.append(vt)

        # per-head per-chunk normalized ctx^T [64, 512]
        ctxT = [
            [
                cxp.tile([64, TCW], BF16, tag=f"cT{h}_{tci}", name=f"cT{h}_{tci}")
                for tci in range(NTC)
            ]
            for h in range(HPC)
        ]
        # ctx psums: one stable bank per head
        cps = [
            ps_c.tile([128, TCW], F32, tag=f"cps{h}", name=f"cps{h}")
            for h in range(HPC)
        ]

        # ---------- emission pieces ----------
        def qk_piece(nch, mt):
            ps = ps_s.tile([128, 2, 512], F32, tag="S")
            for kc in range(NKC):
                nc.tensor.matmul(
                    ps[:, 0, :],
                    lhsT=wqk_sb[kc][:, mt * 128:(mt + 1) * 128],
                    rhs=xv(kc, nch),
                    start=(kc == 0),
                    stop=(kc == NKC - 1),
                )
            nc.vector.tensor_copy(out=qkT[QK_NAMES[mt]][nch][:, :], in_=ps[:, 0, :])

        def v_piece(nch, i):
            tb = 4 * nch + i
            ps = ps_s.tile([128, 2, 512], F32, tag="S")
            for kc in range(NKC):
                nc.tensor.matmul(
                    ps[:, 0, 0:192],
                    lhsT=xv(kc, nch)[:, i * 128:(i + 1) * 128],
                    rhs=wv_sb[kc][:, :],
                    start=(kc == 0),
                    stop=(kc == NKC - 1),
                )
            dst = v4[nch][:, i, :, 0:64]
            src = ps[:, 0, 0:192].rearrange("p (h e) -> p h e", e=64)
            nc.vector.tensor_copy(out=dst, in_=src)

        def proj_pieces(nch, order=(0, 2, 1, 3)):
            # for nch>=1 (fed as extras): mt0 (qT01) and mt2 (qT22) gate the
            # NEXT chunk's first pair; mt1/mt3 (kT chunks) and v only gate
            # its last two pairs. For nch=0 natural order (scores01 pair0
            # needs mt0+mt1 first).
            ps_list = [lambda mt=mt: qk_piece(nch, mt) for mt in order]
            ps_list += [lambda i=i: v_piece(nch, i) for i in range(4)]
            return ps_list

        def scores01(tci, sb):
            """Packed h0/h1 scores for s-block sb -> psum [128, 2, 512]."""
            off = sb * 128 - tci * TCW  # >=0 iff diagonal block
            lo = max(off, 0)
            ps = ps_s.tile([128, 2, 512], F32, tag="S")
            kch, kcol = sb // 4, (sb % 4) * 128
            for hh in range(2):
                psl = slice(hh * 64, (hh + 1) * 64)
                nc.tensor.matmul(
                    ps[:, hh, lo:512],
                    lhsT=qkT["kT01"][kch][psl, kcol:kcol + 128],
                    rhs=qkT["qT01"][tci][psl, lo:512],
                    start=True,
                    stop=True,
                )
            return ps, lo

        def scores2(tci, sb):
            """Packed h2 scores for s-blocks (sb, sb+1) -> psum [128,2,512]."""
            ps = ps_s.tile([128, 2, 512], F32, tag="S")
            los = []
            for j in range(2):
                sbj = sb + j
                off = sbj * 128 - tci * TCW
                lo = max(off, 0)
                los.append(lo)
                kch, kcol = sbj // 4, (sbj % 4) * 128
                psl = slice(j * 64, (j + 1) * 64)
                # full width: the exp reads the whole tile, so every column
                # must be written this generation (masked-out cols get zeroed
                # by the wide zt mask after exp)
                nc.tensor.matmul(
                    ps[:, j, :],
                    lhsT=qkT["kT22"][kch][psl, kcol:kcol + 128],
                    rhs=qkT["qT22"][tci][psl, :],
                    start=True,
                    stop=True,
                )
            return ps, los

        def ctx01(tci, sb, e_t, nsb, lo=0):
            # lo>0 on diagonal blocks: cols [0:lo) of e are memset zeros --
            # skip them (saves tensor streaming cycles; psum cols [0:lo)
            # keep their accumulated value)
            for hh in range(2):
                nc.tensor.matmul(
                    cps[hh][0:66, lo:512],
                    lhsT=v4[sb // 4][:, sb % 4, hh, :],
                    rhs=e_t[:, hh, lo:512],
                    start=(sb == 0),
                    stop=(sb == nsb - 1),
                )

        def ctx2(tci, sb, e_t, nsb, los=(0, 0)):
            for j in range(2):
                sbj = sb + j
                lo = los[j]
                nc.tensor.matmul(
                    cps[2][0:66, lo:512],
                    lhsT=v4[sbj // 4][:, sbj % 4, 2, :],
                    rhs=e_t[:, j, lo:512],
                    start=(sbj == 0),
                    stop=(sbj == nsb - 1),
                )

        cue_t = {}

        def epi_cue(tci, h):
            """Evac ctx+sums for head h (frees the cps bank for the next
            chunk). Emitted in-chunk right after the head's final ctx; at
            the tail the scalar engine does it (it has no exps left)."""
            cue = rpool.tile([66, TCW], F32, tag=f"cue{h}", name=f"cue{tci}_{h}")
            if tci == 3:
                nc.scalar.copy(out=cue[:, :], in_=cps[h][0:66, :])
            else:
                nc.vector.tensor_copy(out=cue[:, :], in_=cps[h][0:66, :])
            cue_t[(tci, h)] = cue

        def epi_fin(tci, h):
            """Normalize into ctxT. Mid-kernel: broadcast sums via a DRAM
            round-trip DMA (zero tensor-engine cost, latency hides under
            attention). Tail (tci=3): the latency is exposed, so use a K=1
            broadcast matmul into the freed cps bank instead (TE is idle)."""
            cue = cue_t.pop((tci, h))
            if tci == 3:
                psB = ps_c.tile([128, TCW], F32, tag=f"cps{h}", name=f"bc{tci}_{h}")
                nc.tensor.matmul(
                    psB[0:64, :],
                    lhsT=ones_row[64:65, :],
                    rhs=cue[64:65, :],
                    start=True,
                    stop=True,
                )
                sb_src = psB[0:64, :]
            else:
                idx = tci * HPC + h
                nc.sync.dma_start(out=srow[idx:idx + 1, :], in_=cue[64:65, :])
                sb_b = rpool.tile([64, TCW], F32, tag=f"sb{h}", name=f"sb{tci}_{h}")
                nc.sync.dma_start(
                    out=sb_b[:, :],
                    in_=srow[idx:idx + 1, :].to_broadcast([64, TCW]),
                )
                sb_src = sb_b[:, :]
            rec = rpool.tile([64, TCW], F32, tag=f"rec{h}", name=f"rec{tci}_{h}")
            nc.vector.reciprocal_approx_fast(out=rec[:, :], in_=sb_src)
            nc.vector.tensor_mul(ctxT[h][tci][:, :], cue[0:64, :], rec[:, :])

        def po_piece(tci, i, tag="O"):
            """Output projection for row block tb = 4*tci + i."""
            tb = 4 * tci + i
            if tag == "O":
                po = ps_o.tile([128, 512], F32, tag="O", name=f"po{tb}")
            else:
                po = ps_c.tile([128, TCW], F32, tag=tag, name=f"po{tb}")
            for h in range(HPC):
                nc.tensor.matmul(
                    po[:, 0:384],
                    lhsT=ctxT[h][tci][:, i * 128:(i + 1) * 128],
                    rhs=wo_sb[h][:, :],
                    start=(h == 0),
                    stop=(h == HPC - 1),
                )
            osb = opool.tile([128, 384], F32, tag="osb", name=f"osb{tb}")
            # tail (tci=3): scalar engine is idle (exps done) and the sync
            # queue is free -- route the evac + store off the busy engines
            if tci == 3:
                # alternate the evac between the two psum-capable engines
                if i % 2 == 0:
                    nc.scalar.copy(out=osb[:, :], in_=po[:, 0:384])
                else:
                    nc.vector.tensor_copy(out=osb[:, :], in_=po[:, 0:384])
                nc.sync.dma_start(out=out[tb * 128:(tb + 1) * 128, :], in_=osb[:, :])
            else:
                nc.vector.tensor_copy(out=osb[:, :], in_=po[:, 0:384])
                nc.gpsimd.dma_start(out=out[tb * 128:(tb + 1) * 128, :], in_=osb[:, :])

        def attn(tci, extras, s_carry):
            """Attention for t-chunk tci; pops `extras` (closures) into the
            emission stream to fill tensor-engine slack. scores01 for each
            pair is hoisted one pair ahead (including across the chunk
            boundary via s_carry) so the scalar engine never waits on TE
            queue position. Returns the next chunk's hoisted first scores."""
            nsb = 4 * tci + 4
            npair = nsb // 2
            ex_i = 0
            slots_left = 3 * npair

            def pop_extras():
                nonlocal ex_i, slots_left
                n_left = len(extras) - ex_i
                k = -(-n_left // max(slots_left, 1))
                slots_left -= 1
                for _ in range(k):
                    if ex_i < len(extras):
                        extras[ex_i]()
                        ex_i += 1

            for p in range(npair):
                sb, sbb = 2 * p, 2 * p + 1
                diag_a = sb >= 4 * tci
                diag_b = sbb >= 4 * tci
                last = p == npair - 1

                # scores h0/h1 for sb (hoisted into the previous pair)
                if s_carry is not None:
                    s_a, lo_a = s_carry
                else:
                    s_a, lo_a = scores01(tci, sb)
                # exp(sb) -- into dedicated diag tile or rotating tile
                if diag_a:
                    e_a = diag01[sb - 4 * tci]
                    nc.scalar.activation(
                        e_a[:, :, lo_a:512], s_a[:, :, lo_a:512], EXPF
                    )
                else:
                    e_a = expp.tile([128, 2, 512], BF16, tag="E")
                    nc.scalar.activation(e_a[:, :, :], s_a[:, :, :], EXPF)
                # scores h0/h1 for sbb (runs on TE while exp(sb) runs on ACT)
                s_b, lo_b = scores01(tci, sbb)
                pop_extras()
                if diag_a:
                    nc.vector.tensor_mul(
                        e_a[:, :, lo_a:lo_a + 128],
                        e_a[:, :, lo_a:lo_a + 128],
                        tri2_sb[:, :, :],
                    )
                ctx01(tci, sb, e_a, nsb, lo=lo_a)
                # exp(sbb)
                if diag_b:
                    e_b = diag01[sbb - 4 * tci]
                    nc.scalar.activation(
                        e_b[:, :, lo_b:512], s_b[:, :, lo_b:512], EXPF
                    )
                else:
                    e_b = expp.tile([128, 2, 512], BF16, tag="E")
                    nc.scalar.activation(e_b[:, :, :], s_b[:, :, :], EXPF)
                # h2 scores for the pair (TE during exp(sbb))
                s_2, los2 = scores2(tci, sb)
                # hoist the NEXT pair's (or next chunk's) h0/h1 scores
                if not last:
                    s_carry = scores01(tci, sb + 2)
                elif tci + 1 < NTC:
                    s_carry = scores01(tci + 1, 0)
                else:
                    s_carry = None
                pop_extras()
                if diag_b:
                    nc.vector.tensor_mul(
                        e_b[:, :, lo_b:lo_b + 128],
                        e_b[:, :, lo_b:lo_b + 128],
                        tri2_sb[:, :, :],
                    )
                ctx01(tci, sbb, e_b, nsb, lo=lo_b)
                if last and tci < 3:
                    epi_cue(tci, 0)
                    epi_cue(tci, 1)
                # exp h2 (full width; stale prefix cols get masked below)
                e_2 = expp.tile([128, 2, 512], BF16, tag="E")
                nc.scalar.activation(e_2[:, :, :], s_2[:, :, :], EXPF)
                if last and tci == 3:
                    # scalar-engine cues must queue AFTER the final exp
                    epi_cue(tci, 0)
                    epi_cue(tci, 1)
                for j in range(2):
                    if sb + j >= 4 * tci:  # diagonal: wide zt mask
                        w = los2[j] + 128
                        nc.vector.tensor_mul(
                            e_2[:, j, 0:w], e_2[:, j, 0:w], zt_sb[:, 512 - w:512]
                        )
                ctx2(tci, sb, e_2, nsb, los=[
                    los2[j] if sb + j >= 4 * tci else 0 for j in range(2)
                ])
                if last:
                    epi_cue(tci, 2)
                pop_extras()
            while ex_i < len(extras):  # any stragglers
                extras[ex_i]()
                ex_i += 1
            return s_carry

        # ---------- top-level schedule ----------
        for piece in proj_pieces(0, order=(0, 1, 2, 3)):
            piece()

        def fin_pieces(tci):
            return [lambda h=h: epi_fin(tci, h) for h in range(HPC)]

        def po_pieces(tci):
            return [lambda i=i: po_piece(tci, i) for i in range(4)]

        sc = attn(0, proj_pieces(1), None)
        sc = attn(1, fin_pieces(0) + po_pieces(0) + proj_pieces(2), sc)
        sc = attn(2, fin_pieces(1) + po_pieces(1) + proj_pieces(3), sc)
        sc = attn(3, fin_pieces(2) + po_pieces(2), sc)
        # tail: finish chunk 3's normalize, then pipeline the last 4 output
        # projections across the freed ctx psum banks + the O bank
        for h in range(HPC):
            epi_fin(3, h)
        for i, tg in enumerate(["O", "cps0", "cps1", "cps2"]):
            po_piece(3, i, tag=tg)

    return nc


def get_nc():
    global _CACHED_NC
    if _CACHED_NC is None:
        nc = build_nc()
        nc.finalize()
        _CACHED_NC = nc
    return _CACHED_NC


def make_core_inputs(x, Wq, bq, Wk, bk, Wv, bv, Wo, bo):
    """Host-side shard prep. Returns (in_maps, host_add) where host_add[384]
    is added to every output row (exact fold of bv/bo)."""
    scale = 1.0 / math.sqrt(D)
    assert np.all(bq == 0.0) and np.all(bk == 0.0), "kernel assumes bq=bk=0"
    host_add = (bv.astype(np.float64) @ Wo.astype(np.float64) + bo).astype(np.float32)

    si = np.arange(128)[:, None]
    tj = np.arange(128)[None, :]
    tri = (si <= tj).astype(np.float32)
    zt = np.zeros((128, 512), dtype=np.float32)
    zt[:, 384:512] = tri
    tri2 = np.concatenate([tri, tri], axis=1)

    in_maps = []
    for core in range(NCORES):
        b = core // 2
        h0 = HPC * (core % 2)  # first head (0 or 3)
        cs = slice(h0 * D, (h0 + HPC) * D)
        wq_s = (Wq[:, cs] * scale).astype(np.float32)
        wk_s = Wk[:, cs].astype(np.float32)
        wqk = np.concatenate(
            [
                wq_s[:, 0:128],
                wk_s[:, 0:128],
                np.tile(wq_s[:, 128:192], (1, 2)),
                np.tile(wk_s[:, 128:192], (1, 2)),
            ],
            axis=1,
        )
        in_maps.append(
            {
                "xt": np.ascontiguousarray(x[b].T).astype(BF16NP),
                "wqk": np.ascontiguousarray(wqk).astype(BF16NP),
                "wv": np.ascontiguousarray(Wv[:, cs]).astype(BF16NP),
                "wo": np.ascontiguousarray(Wo[cs, :]).astype(BF16NP),
                "zt": zt.astype(BF16NP),
                "tri2": tri2.astype(BF16NP),
            }
        )
    return in_maps, host_add


def kernel(x, Wq, bq, Wk, bk, Wv, bv, Wo, bo, _trace=False):
    x = np.asarray(x, dtype=np.float32)
    Wq, bq = np.asarray(Wq, np.float32), np.asarray(bq, np.float32)
    Wk, bk = np.asarray(Wk, np.float32), np.asarray(bk, np.float32)
    Wv, bv = np.asarray(Wv, np.float32), np.asarray(bv, np.float32)
    Wo, bo = np.asarray(Wo, np.float32), np.asarray(bo, np.float32)

    from concourse.bass_utils import run_bass_kernel_spmd

    nc = get_nc()
    in_maps, host_add = make_core_inputs(x, Wq, bq, Wk, bk, Wv, bv, Wo, bo)
    res = run_bass_kernel_spmd(
        nc, in_maps, core_ids=list(range(NCORES)), trace=_trace
    )
    out = np.empty((B, T, C), dtype=np.float32)
    for b in range(B):
        out[b] = res.results[2 * b]["out"] + res.results[2 * b + 1]["out"] + host_add
    if _trace:
        return out, res
    return out
